# revision 11
# baseline (speedup 1.0000x reference)
"""Trainium2 Bass kernel for nn_DWT_1D: db4 DWT along the last axis.

Reference computes lo = einsum('ncl,kl->nck', x, matrix_low) (and hi with
matrix_high) where matrix_low/high are banded strided matrices: each output
k depends on 8 input elements x[2k-3 : 2k+5].

Strategy (data-parallel over N, 2 batch rows = 128 (n,c) rows per core),
fp16 end-to-end (tolerance is 2e-2 rel fro; fp16 gives ~3e-4):

  - The host pre-transposes each core's input into 64 non-overlapping
    128-column blocks xt[l, b*128+r] = x[r, 128*b+l] (fp16), with the
    banded weight blocks [W0 | WL | WR] prepended, so there are no on-chip
    transposes and input DMA is the bare 2.1 MB/core.
  - Output block b (64 cols per filter, both filters at once) is one K=128
    matmul of block b against the shift-invariant banded weight block
    W0 [128, 2, 64], plus two 4-column seam matmuls (WL reads block b-1,
    WR reads block b+1) accumulated into the same PSUM bank.  Skipping the
    seams at b=0 / b=63 reproduces the reference's edge truncation exactly.
    PE cost: 64*(128+4+4) cycles ~= 3.6 us at fp16 rate.
  - 16 groups of 4 blocks, one PSUM bank each (4 banks rotating).  PSUM
    (fp32) -> SBUF (fp16) drain copies alternate ScalarE/VectorE; output
    DMAs go out every 2 groups so each is gated on only two copies.
  - Raw bass (no TileContext): hand-managed semaphores remove the Tile
    entry barrier and most of its exit cascade.  The schedule keeps the
    DMA device (the roofline: ~4.2 MB in+out fp16 at 360 GB/s = 11.7 us)
    saturated with zero idle gaps; total = 0.7us framework preamble
    + 1.3us first-DMA pipe latency + 11.7us DMA + 0.9us final DMA
    semaphore + a short exit barrier.

  SP   : 6 input DMAs -> per-slab copy waits -> 9 output DMAs -> final
         DMA-landed wait (kernel sems are re-cleared by the framework
         preamble of the next execution, so no exit sem hygiene needed).
  PE   : warm matmuls (garbage data, never read) to start the HAM clock
         ramp, then per group: wait input chunk sem + psum-bank-reuse sem,
         12-14 banded matmuls, inc pe_sem.
  ACT  : copies even groups PSUM->slab (fp32->fp16), inc act_sem.
  DVE  : copies odd groups, inc dve_sem.
"""

import numpy as np

import concourse.bacc as bacc
import concourse.bass as bass
import concourse.mybir as mybir

FP16 = mybir.dt.float16
FP32 = mybir.dt.float32
P = 128
LIN = 8192
LOUT = 4096
NCORES = 8
NB = 64
BPG = 4
NG = NB // BPG
GW = BPG * 64
WTW = 2 * 64 + 4 + 4
IN_CHUNKS = [8, 8, 10, 12, 13, 13]
SLAB_GROUPS = [2, 2, 2, 2, 2, 2, 2, 1, 1]
NWARM = 14

assert sum(IN_CHUNKS) == NB
assert sum(SLAB_GROUPS) == NG

LAST_RESULTS = None


def build_nc() -> bass.Bass:
    nc = bacc.Bacc("TRN2")
    xt = nc.dram_tensor("xt", [P, WTW + NB * P], FP16, kind="ExternalInput")
    out = nc.dram_tensor("out", [P, 2, LOUT], FP16, kind="ExternalOutput")

    # chunk boundaries (in xt columns) and block -> chunk index
    cum = [0]
    for nblk in IN_CHUNKS:
        cum.append(cum[-1] + nblk)
    blk_chunk = {}
    for j in range(len(IN_CHUNKS)):
        for b in range(cum[j], cum[j + 1]):
            blk_chunk[b] = j

    def chunk_for_group(g):
        """Last chunk needed by group g (incl. the right-seam block)."""
        return blk_chunk[min(BPG * g + BPG, NB - 1)]

    gs0 = [0]
    for s in SLAB_GROUPS:
        gs0.append(gs0[-1] + s)

    with (
        nc.sbuf_tensor("xt_sb", [P, WTW + NB * P], FP16) as xt_sb,
        nc.sbuf_tensor("slab_sb", [P, 2, LOUT], FP16) as slab_sb,
        nc.sbuf_tensor("warm_sb", [P, P], FP16) as warm_sb,
        nc.psum_tensor("warm_ps", [P, P], FP32) as warm_ps,
        nc.psum_tensor("gt0", [P, 2, GW], FP32) as gt0,
        nc.psum_tensor("gt1", [P, 2, GW], FP32) as gt1,
        nc.psum_tensor("gt2", [P, 2, GW], FP32) as gt2,
        nc.psum_tensor("gt3", [P, 2, GW], FP32) as gt3,
        nc.semaphore("dma_sem") as dma_sem,
        nc.semaphore("odma_sem") as odma_sem,
        nc.semaphore("pe_sem") as pe_sem,
        nc.semaphore("act_sem") as act_sem,
        nc.semaphore("dve_sem") as dve_sem,
        nc.Block() as block,
    ):
        gts = [gt0, gt1, gt2, gt3]
        w0 = xt_sb[:, 0:128].rearrange("p (f m) -> p f m", f=2)
        wl = xt_sb[:, 128:132].rearrange("p (f m) -> p f m", f=2)
        wr = xt_sb[:, 132:136].rearrange("p (f m) -> p f m", f=2)

        def blk(b):
            return xt_sb[:, WTW + b * P : WTW + (b + 1) * P]

        @block.sync
        def _(sync):
            c0 = 0
            for j, nblk in enumerate(IN_CHUNKS):
                lo = 0 if j == 0 else WTW + c0 * P
                hi = WTW + (c0 + nblk) * P
                sync.dma_start(xt_sb[:, lo:hi], xt[:, lo:hi]).then_inc(
                    dma_sem, 16
                )
                c0 += nblk
            for m in range(len(SLAB_GROUPS)):
                gend = gs0[m + 1]
                sync.wait_ge(act_sem, (gend + 1) // 2)
                sync.wait_ge(dve_sem, gend // 2)
                d0 = gs0[m] * GW
                d1 = gend * GW
                sync.dma_start(
                    out[:, :, d0:d1], slab_sb[:, :, d0:d1]
                ).then_inc(odma_sem, 16)
            # hold the SP program open until the last output write is
            # confirmed landed in DRAM (the bass construction-time preamble
            # of the next execution re-clears all kernel sems, so no
            # explicit sem hygiene is needed here)
            sync.wait_ge(odma_sem, 16 * len(SLAB_GROUPS))

        @block.tensor
        def _(pe):
            # HAM warmup on garbage data (never read back)
            for _ in range(NWARM):
                nc.tensor.matmul(warm_ps[:], warm_sb[:], warm_sb[:],
                                 start=True, stop=True)
            for g in range(NG):
                pe.wait_ge(dma_sem, 16 * (chunk_for_group(g) + 1))
                if g >= 4:
                    # psum bank g%4 was drained by the copy of group g-4
                    prev = g - 4
                    sem = act_sem if prev % 2 == 0 else dve_sem
                    pe.wait_ge(sem, prev // 2 + 1)
                gt = gts[g % 4]
                last = None
                for i in range(BPG):
                    b = BPG * g + i
                    r0 = 64 * i
                    last = nc.tensor.matmul(
                        gt[:, :, r0 : r0 + 64], blk(b), w0,
                        start=(i == 0), stop=False,
                    )
                    if b > 0:
                        last = nc.tensor.matmul(
                            gt[:, :, r0 : r0 + 2], blk(b - 1), wl,
                            start=False,
                            stop=(i == BPG - 1 and b == NB - 1),
                        )
                    if b < NB - 1:
                        last = nc.tensor.matmul(
                            gt[:, :, r0 + 62 : r0 + 64], blk(b + 1), wr,
                            start=False, stop=(i == BPG - 1),
                        )
                last.then_inc(pe_sem, 1)

        @block.scalar
        def _(act):
            for g in range(0, NG, 2):
                act.wait_ge(pe_sem, g + 1)
                nc.scalar.copy(
                    slab_sb[:, :, g * GW : (g + 1) * GW], gts[g % 4][:]
                ).then_inc(act_sem, 1)

        @block.vector
        def _(dve):
            for g in range(1, NG, 2):
                dve.wait_ge(pe_sem, g + 1)
                nc.vector.tensor_copy(
                    slab_sb[:, :, g * GW : (g + 1) * GW], gts[g % 4][:]
                ).then_inc(dve_sem, 1)

    nc.compile()
    return nc


_NC_CACHE = None


def _get_nc() -> bass.Bass:
    global _NC_CACHE
    if _NC_CACHE is None:
        _NC_CACHE = build_nc()
    return _NC_CACHE


def _build_weights(ml, mh):
    b0 = 30
    W0 = np.zeros((P, 2, 64), dtype=np.float16)
    W0[:, 0, :] = ml[64 * b0 : 64 * b0 + 64, 128 * b0 : 128 * b0 + 128].T
    W0[:, 1, :] = mh[64 * b0 : 64 * b0 + 64, 128 * b0 : 128 * b0 + 128].T
    WL = np.zeros((P, 2, 2), dtype=np.float16)
    WL[:, 0, :] = ml[64 * b0 : 64 * b0 + 2, 128 * (b0 - 1) : 128 * b0].T
    WL[:, 1, :] = mh[64 * b0 : 64 * b0 + 2, 128 * (b0 - 1) : 128 * b0].T
    WR = np.zeros((P, 2, 2), dtype=np.float16)
    WR[:, 0, :] = ml[64 * b0 + 62 : 64 * b0 + 64,
                     128 * (b0 + 1) : 128 * (b0 + 2)].T
    WR[:, 1, :] = mh[64 * b0 + 62 : 64 * b0 + 64,
                     128 * (b0 + 1) : 128 * (b0 + 2)].T
    return np.concatenate(
        [W0.reshape(P, 128), WL.reshape(P, 4), WR.reshape(P, 4)], axis=1
    )


def kernel(input, matrix_low, matrix_high, *, trace=False, tmpdir=None):
    global LAST_RESULTS
    from concourse.bass_utils import run_bass_kernel_spmd

    x = np.asarray(input, dtype=np.float32)
    ml = np.asarray(matrix_low, dtype=np.float32)
    mh = np.asarray(matrix_high, dtype=np.float32)
    assert x.shape == (16, 64, LIN), x.shape

    wts_np = _build_weights(ml, mh)

    nc = _get_nc()
    in_maps = []
    for d in range(NCORES):
        x128 = x[2 * d : 2 * d + 2].reshape(P, LIN).astype(np.float16)
        xt_np = np.empty((P, WTW + NB * P), dtype=np.float16)
        xt_np[:, :WTW] = wts_np
        xt_np[:, WTW:] = x128.reshape(P, NB, P).transpose(2, 1, 0).reshape(
            P, NB * P
        )
        in_maps.append({"xt": xt_np})

    res = run_bass_kernel_spmd(
        nc, in_maps, core_ids=list(range(NCORES)), trace=trace, tmpdir=tmpdir
    )
    LAST_RESULTS = res
    both = np.stack(
        [r["out"].astype(np.float32).reshape(2, 64, 2, LOUT) for r in res.results]
    )
    lo = np.ascontiguousarray(both[:, :, :, 0, :].reshape(16, 64, LOUT))
    hi = np.ascontiguousarray(both[:, :, :, 1, :].reshape(16, 64, LOUT))
    return lo, hi
